# revision 3
# baseline (speedup 1.0000x reference)
DEBUG = False

"""DKVMN forward kernel v2 for 8 Trainium2 NeuronCores (Bass/Tile).

Strategy vs baseline: fp16 throughout; scan state in (m,b)-partition layout
so the recurrence's w-multiplies become per-partition scalars (fast DVE/Pool
tensor_scalar ops), updates are 4x-mode fp16 tensor_tensor ops, and the read
r_t = w_t^T Mv_t runs on the otherwise-idle PE as w-weighted block-diag
selector matmuls accumulating into PSUM (output directly feature-major for
the final f-matmul — no reads DRAM bounce).

Per-core layout:
  * tokens: i = 256*b + t; token-major tiles [128, NG, D], p=t%128, g=2b+t//128.
  * all main matmuls produce TOKEN-major output (stationary = feature-major
    activations, moving = weight matrix), so no post-matmul transposes.
  * scan partitions p = 8*mq + b (mq in [0,16), b in [0,8)); m = 16*q + mq,
    q in [0,4).  State Mv [128, 4, 256] fp16, ping-pong buffered.
  * per step t: G_q = 1 - w[q]*e_t (DVE ts), H_q = w[q]*a_t (Pool ts),
    Mv' = Mv*G + H (2 DVE 4x tt); reads on PE: stat=Mv d-half, mov=S_t
    (S_t[p,b'] = w_t[p]*delta_{b,b'}), PSUM [d_half, (t,b)] -> rT_sb fp16
    drained by Act every 64 steps.
"""

import numpy as np

import concourse.bass as bass
import concourse.mybir as mybir
import concourse.tile as tile
from concourse import bacc
from concourse.bass_utils import run_bass_kernel_spmd

F32 = mybir.dt.float32
F16 = mybir.dt.float16
I32 = mybir.dt.int32
ALU = mybir.AluOpType
ACTF = mybir.ActivationFunctionType
AXL = mybir.AxisListType

B, T, D, M, V = 64, 200, 256, 64, 1000
NCORES = 8
BL = B // NCORES          # 8 batches per core
TP = 256                  # padded time for token indexing
NTOK = BL * TP            # 2048 tokens / core
NG = NTOK // 128          # 16 token groups
NQ = 4                    # m-quarters: m = 16*q + mq
TC = 50                   # e/a t-chunk streamed from DRAM during scan
RB = 64                   # reads PSUM block (steps per bank)


def build_nc():
    nc = bacc.Bacc("TRN2", target_bir_lowering=False, enable_partition_id=False)

    # ---- external inputs -------------------------------------------------
    idx_k_d = nc.dram_tensor("idx_k", [128, NG], I32, kind="ExternalInput")
    idx_v_d = nc.dram_tensor("idx_v", [128, NG], I32, kind="ExternalInput")
    k_emb_d = nc.dram_tensor("k_emb", [V, D], F16, kind="ExternalInput")
    v_emb_d = nc.dram_tensor("v_emb", [2 * V, D], F16, kind="ExternalInput")
    mkT_d = nc.dram_tensor("mkT", [D, M], F16, kind="ExternalInput")
    ewT_d = nc.dram_tensor("ewT", [D, D], F16, kind="ExternalInput")
    awT_d = nc.dram_tensor("awT", [D, D], F16, kind="ExternalInput")
    fwT_d = nc.dram_tensor("fwT", [2 * D, D], F16, kind="ExternalInput")
    pw_rep_d = nc.dram_tensor("pw_rep", [128, D], F16, kind="ExternalInput")
    eb_d = nc.dram_tensor("eb", [1, D], F16, kind="ExternalInput")
    ab_d = nc.dram_tensor("ab", [1, D], F16, kind="ExternalInput")
    fb_d = nc.dram_tensor("fb", [1, D], F16, kind="ExternalInput")
    pb_rep_d = nc.dram_tensor("pb_rep", [128, 1], F32, kind="ExternalInput")
    mv0_d = nc.dram_tensor("mv0", [128, NQ * D], F16, kind="ExternalInput")
    mask8_d = nc.dram_tensor("mask8", [128, BL], F16, kind="ExternalInput")
    id_d = nc.dram_tensor("ident", [128, 128], F16, kind="ExternalInput")
    ones_d = nc.dram_tensor("ones_row", [1, 128], F16, kind="ExternalInput")

    # ---- DRAM scratch ----------------------------------------------------
    ea_d = nc.dram_tensor("ea_x", [BL, TP, 2, D], F16, kind="Internal")
    w8t_d = nc.dram_tensor("w8t_x", [BL, M, T], F16, kind="Internal")

    # ---- output ----------------------------------------------------------
    p_out_d = nc.dram_tensor("p_out", [128, NG], F32, kind="ExternalOutput")
    if DEBUG:
        dbg_kT = nc.dram_tensor("dbg_kT", [128, 2 * NTOK], F32, kind="ExternalOutput")
        dbg_vT = nc.dram_tensor("dbg_vT", [128, 2 * NTOK], F32, kind="ExternalOutput")
        dbg_w = nc.dram_tensor("dbg_w", [128, NG * M], F32, kind="ExternalOutput")
        dbg_ea = nc.dram_tensor("dbg_ea", [128, NG * 2 * D], F32, kind="ExternalOutput")
        dbg_wc = nc.dram_tensor("dbg_wc", [128, NQ * T], F32, kind="ExternalOutput")
        dbg_sall = nc.dram_tensor("dbg_sall", [128, T * NQ * BL], F32, kind="ExternalOutput")
        dbg_rT = nc.dram_tensor("dbg_rT", [128, 2 * NTOK], F32, kind="ExternalOutput")
        dbg_mv = nc.dram_tensor("dbg_mv", [128, NQ * D], F32, kind="ExternalOutput")
        dbg_ech = nc.dram_tensor("dbg_ech", [128, 25 * D], F32, kind="ExternalOutput")
        dbg_G = nc.dram_tensor("dbg_G", [128, NQ * D], F32, kind="ExternalOutput")
        dbg_H = nc.dram_tensor("dbg_H", [128, NQ * D], F32, kind="ExternalOutput")
        dbg_mv1 = nc.dram_tensor("dbg_mv1", [128, NQ * D], F32, kind="ExternalOutput")

    with tile.TileContext(nc) as tc:
        with tc.tile_pool(name="persist", bufs=1) as pp:
            # weights / constants
            mkT_sb = pp.tile([128, 2, M], F16)
            nc.sync.dma_start(
                out=mkT_sb, in_=mkT_d.ap().rearrange("(h p) m -> p h m", p=128))
            ewT_sb = pp.tile([128, 2, D], F16)
            nc.sync.dma_start(
                out=ewT_sb, in_=ewT_d.ap().rearrange("(h p) n -> p h n", p=128))
            awT_sb = pp.tile([128, 2, D], F16)
            nc.sync.dma_start(
                out=awT_sb, in_=awT_d.ap().rearrange("(h p) n -> p h n", p=128))
            fwT_sb = pp.tile([128, 4, D], F16)
            nc.sync.dma_start(
                out=fwT_sb, in_=fwT_d.ap().rearrange("(q p) n -> p q n", p=128))
            pw_rep_sb = pp.tile([128, D], F16)
            nc.sync.dma_start(out=pw_rep_sb, in_=pw_rep_d.ap())
            eb_sb = pp.tile([1, D], F16)
            nc.sync.dma_start(out=eb_sb, in_=eb_d.ap())
            ab_sb = pp.tile([1, D], F16)
            nc.sync.dma_start(out=ab_sb, in_=ab_d.ap())
            fb_sb = pp.tile([1, D], F16)
            nc.sync.dma_start(out=fb_sb, in_=fb_d.ap())
            pb_sb = pp.tile([128, 1], F32)
            nc.sync.dma_start(out=pb_sb, in_=pb_rep_d.ap())
            mask8_sb = pp.tile([128, BL], F16)
            nc.sync.dma_start(out=mask8_sb, in_=mask8_d.ap())
            id_sb = pp.tile([128, 128], F16)
            nc.sync.dma_start(out=id_sb, in_=id_d.ap())
            ones_sb = pp.tile([1, 128], F16)
            nc.sync.dma_start(out=ones_sb, in_=ones_d.ap())

            # long-lived activations
            kT_sb = pp.tile([128, 2, NTOK], F16)     # k feature-major
            rT_sb = pp.tile([128, 2, NTOK], F16)     # reads feature-major
            nc.vector.memset(rT_sb, 0.0)
            w_cols = pp.tile([128, NQ, T], F16)      # w[b, 16q+mq, t] at p=8mq+b
            negw = pp.tile([128, NQ, T], F32)        # -w (ts scalars)
            wpos = pp.tile([128, NQ, T], F32)        # +w (ts scalars)
            s_all = pp.tile([128, T, NQ, BL], F16)   # PE read selectors

            # =========== PRE: gather, transpose, matmuls, softmax =========
            with (
                tc.tile_pool(name="gath", bufs=2) as gp,
                tc.tile_pool(name="pre", bufs=1) as fp,
                tc.tile_pool(name="preps", bufs=2, space="PSUM") as psp,
            ):
                idxk_sb = fp.tile([128, NG], I32)
                nc.sync.dma_start(out=idxk_sb, in_=idx_k_d.ap())
                idxv_sb = fp.tile([128, NG], I32)
                nc.sync.dma_start(out=idxv_sb, in_=idx_v_d.ap())

                vT_sb = fp.tile([128, 2, NTOK], F16)

                # gather + transpose to feature-major (stationaries)
                for idx_sb, emb_d, dst in (
                    (idxk_sb, k_emb_d, kT_sb),
                    (idxv_sb, v_emb_d, vT_sb),
                ):
                    tok = gp.tile([128, NG, D], F16, tag="tok")
                    for g in range(NG):
                        nc.gpsimd.indirect_dma_start(
                            out=tok[:, g, :],
                            out_offset=None,
                            in_=emb_d.ap(),
                            in_offset=bass.IndirectOffsetOnAxis(
                                ap=idx_sb[:, g : g + 1], axis=0),
                        )
                    for g in range(NG):
                        for h in range(2):
                            tp_ps = psp.tile([128, 128], F16, tag="tp")
                            nc.tensor.transpose(
                                tp_ps, tok[:, g, 128 * h : 128 * (h + 1)], id_sb)
                            nc.scalar.copy(
                                dst[:, h, 128 * g : 128 * (g + 1)], tp_ps)

                # scores token-major: s[tok, m] = k @ Mk^T; softmax over m
                wtok_sb = fp.tile([128, NG, M], F16)
                for g in range(NG):
                    sc_ps = psp.tile([128, M], F32, tag="sc")
                    for h in range(2):
                        nc.tensor.matmul(
                            sc_ps,
                            kT_sb[:, h, 128 * g : 128 * (g + 1)],
                            mkT_sb[:, h, :],
                            start=(h == 0), stop=(h == 1))
                    nc.scalar.activation(wtok_sb[:, g, :], sc_ps, ACTF.Exp)
                ssum = fp.tile([128, NG], F32)
                nc.vector.tensor_reduce(ssum, wtok_sb, axis=AXL.X, op=ALU.add)
                srec = fp.tile([128, NG], F32)
                nc.vector.reciprocal(srec, ssum)
                nc.vector.tensor_tensor(
                    wtok_sb, wtok_sb,
                    srec.unsqueeze(2).broadcast_to([128, NG, M]),
                    op=ALU.mult)

                # w -> m-major wT [64, NTOK] via PE transposes
                wT_sb = fp.tile([M, NTOK], F16)
                for g in range(NG):
                    wt_ps = psp.tile([M, 128], F16, tag="wt")
                    nc.tensor.transpose(
                        wt_ps, wtok_sb[:, g, :], id_sb)
                    nc.scalar.copy(wT_sb[:, 128 * g : 128 * (g + 1)], wt_ps)
                # bounce w to DRAM [b, m, t]
                for b in range(BL):
                    nc.sync.dma_start(
                        out=w8t_d.ap()[b : b + 1, :, :],
                        in_=wT_sb[:, TP * b : TP * b + T])
                # load w_cols [p=8mq+b, q, t]: m-axis viewed as (q, mq)
                w8t_q = w8t_d.ap().rearrange("b (q mq) t -> b q mq t", q=NQ)
                for mq in range(16):
                    nc.sync.dma_start(
                        out=w_cols[8 * mq : 8 * mq + 8, :, :],
                        in_=w8t_q[:, :, mq, :])
                nc.vector.tensor_scalar(
                    negw, w_cols, -1.0, 0.0, op0=ALU.mult, op1=ALU.add)
                nc.scalar.copy(wpos, w_cols)
                # selectors: s_all[p, t, q, b'] = w_cols[p,q,t] * mask8[p,b']
                nc.gpsimd.tensor_tensor(
                    s_all,
                    w_cols.rearrange("p q t -> p t q")
                        .unsqueeze(3).broadcast_to([128, T, NQ, BL]),
                    mask8_sb.unsqueeze(1).unsqueeze(1)
                        .broadcast_to([128, T, NQ, BL]),
                    op=ALU.mult)

                # e/a token-major: e = sigmoid(v @ eW^T + eb), a = tanh(...)
                ea_tok = fp.tile([128, NG, 2, D], F16)
                for g in range(NG):
                    for j, (wsb, bsb, func) in enumerate((
                        (ewT_sb, eb_sb, ACTF.Sigmoid),
                        (awT_sb, ab_sb, ACTF.Tanh),
                    )):
                        ea_ps = psp.tile([128, D], F32, tag="ea")
                        for h in range(2):
                            nc.tensor.matmul(
                                ea_ps,
                                vT_sb[:, h, 128 * g : 128 * (g + 1)],
                                wsb[:, h, :],
                                start=(h == 0), stop=False)
                        nc.tensor.matmul(
                            ea_ps, ones_sb, bsb,
                            start=False, stop=True)
                        nc.scalar.activation(ea_tok[:, g, j, :], ea_ps, func)
                # bounce e/a to DRAM token-major [b, t, {e,a}, d]
                for g in range(NG):
                    b, th = g // 2, g % 2
                    tlen = 128 if th == 0 else T - 128
                    nc.sync.dma_start(
                        out=ea_d.ap()[b : b + 1, 128 * th : 128 * th + tlen, :, :],
                        in_=ea_tok[0:tlen, g, :, :])

                if DEBUG:
                    dk = fp.tile([128, 2, NTOK], F32)
                    nc.vector.tensor_copy(dk, kT_sb)
                    nc.sync.dma_start(out=dbg_kT.ap(), in_=dk.rearrange("p a b -> p (a b)"))
                    dv = fp.tile([128, 2, NTOK], F32)
                    nc.vector.tensor_copy(dv, vT_sb)
                    nc.sync.dma_start(out=dbg_vT.ap(), in_=dv.rearrange("p a b -> p (a b)"))
                    dw = fp.tile([128, NG, M], F32)
                    nc.vector.tensor_copy(dw, wtok_sb)
                    nc.sync.dma_start(out=dbg_w.ap(), in_=dw.rearrange("p a b -> p (a b)"))
                    dea = fp.tile([128, NG, 2, D], F32)
                    nc.vector.tensor_copy(dea, ea_tok)
                    nc.sync.dma_start(out=dbg_ea.ap(), in_=dea.rearrange("p a b c -> p (a b c)"))
                    dwc = fp.tile([128, NQ, T], F32)
                    nc.vector.tensor_copy(dwc, w_cols)
                    nc.sync.dma_start(out=dbg_wc.ap(), in_=dwc.rearrange("p a b -> p (a b)"))
                    dsa = fp.tile([128, T, NQ, BL], F32)
                    nc.vector.tensor_copy(dsa, s_all)
                    nc.sync.dma_start(out=dbg_sall.ap(), in_=dsa.rearrange("p a b c -> p (a b c)"))

            # ================= SCAN ======================================
            with (
                tc.tile_pool(name="scstat", bufs=1) as sp,
                tc.tile_pool(name="scea", bufs=2) as cp,
                tc.tile_pool(name="scgh", bufs=2) as ghp,
                tc.tile_pool(name="scps", bufs=2, space="PSUM") as rpp,
            ):
                mv_a = sp.tile([128, NQ, D], F16)
                mv_b = sp.tile([128, NQ, D], F16)
                mv = [mv_a, mv_b]
                nc.sync.dma_start(
                    out=mv[0].rearrange("p q d -> p (q d)"), in_=mv0_d.ap())

                ea_ch = None
                r_ps = [None, None]
                for t in range(T):
                    c, tc_off = divmod(t, TC)
                    if tc_off == 0:
                        ea_ch = cp.tile([128, TC, 2, D], F16, tag="each")
                        for mq in range(16):
                            nc.sync.dma_start(
                                out=ea_ch[8 * mq : 8 * mq + 8, :, :, :],
                                in_=ea_d.ap()[:, TC * c : TC * (c + 1), :, :])
                    rb, rb_off = divmod(t, RB)
                    if rb_off == 0:
                        for h in range(2):
                            r_ps[h] = rpp.tile(
                                [128, RB, BL], F32, tag=f"rps{h}",
                                name=f"rps{h}")

                    cur, nxt = mv[t % 2], mv[(t + 1) % 2]

                    # PE reads: r_ps[h][:, t%RB, b'] += Mv[:,q,dh]^T @ S_t
                    for h in range(2):
                        for q in range(NQ):
                            nc.tensor.matmul(
                                r_ps[h][:, rb_off, :],
                                cur[:, q, 128 * h : 128 * (h + 1)],
                                s_all[:, t, q, :],
                                start=(q == 0), stop=(q == NQ - 1))

                    if t < T - 1:
                        # G = 1 - w*e (DVE ts), H = w*a (Pool ts)
                        G = ghp.tile([128, NQ, D], F16, tag="G")
                        H = ghp.tile([128, NQ, D], F16, tag="H")
                        for q in range(NQ):
                            nc.vector.tensor_scalar(
                                G[:, q, :], ea_ch[:, tc_off, 0, :],
                                negw[:, q, t : t + 1], 1.0,
                                op0=ALU.mult, op1=ALU.add)
                            nc.gpsimd.tensor_scalar(
                                H[:, q, :], ea_ch[:, tc_off, 1, :],
                                wpos[:, q, t : t + 1], 0.0,
                                op0=ALU.mult, op1=ALU.add)
                        nc.vector.tensor_tensor(nxt, cur, G, op=ALU.mult)
                        nc.vector.tensor_tensor(nxt, nxt, H, op=ALU.add)
                        if DEBUG and t == 0:
                            dG = sp.tile([128, NQ, D], F32)
                            nc.vector.tensor_copy(dG, G)
                            nc.sync.dma_start(out=dbg_G.ap(), in_=dG.rearrange("p a b -> p (a b)"))
                            dH = sp.tile([128, NQ, D], F32)
                            nc.vector.tensor_copy(dH, H)
                            nc.sync.dma_start(out=dbg_H.ap(), in_=dH.rearrange("p a b -> p (a b)"))
                            dm1 = sp.tile([128, NQ, D], F32)
                            nc.vector.tensor_copy(dm1, nxt)
                            nc.sync.dma_start(out=dbg_mv1.ap(), in_=dm1.rearrange("p a b -> p (a b)"))

                    # drain reads PSUM block
                    if rb_off == min(RB, T - RB * rb) - 1:
                        t0 = RB * rb
                        blen = rb_off + 1
                        for h in range(2):
                            nc.scalar.copy(
                                rT_sb.rearrange(
                                    "p h (e t) -> p h e t", e=BL)[
                                    :, h, :, t0 : t0 + blen],
                                r_ps[h].rearrange(
                                    "p t e -> p e t")[:, :, 0:blen])

                if DEBUG:
                    dmv = sp.tile([128, NQ, D], F32)
                    nc.vector.tensor_copy(dmv, mv[(T - 1) % 2])
                    nc.sync.dma_start(out=dbg_mv.ap(), in_=dmv.rearrange("p a b -> p (a b)"))
                    drt = sp.tile([128, 2, NTOK], F32)
                    nc.vector.tensor_copy(drt, rT_sb)
                    nc.sync.dma_start(out=dbg_rT.ap(), in_=drt.rearrange("p a b -> p (a b)"))

            # ================= POST: f, p ================================
            with (
                tc.tile_pool(name="post", bufs=2) as qp,
                tc.tile_pool(name="postps", bufs=2, space="PSUM") as qpp,
            ):
                p_sb = qp.tile([128, NG], F32, bufs=1)
                quarters = (
                    rT_sb[:, 0, :], rT_sb[:, 1, :],
                    kT_sb[:, 0, :], kT_sb[:, 1, :],
                )
                for g in range(NG):
                    f_ps = qpp.tile([128, D], F32, tag="f")
                    for qi in range(4):
                        nc.tensor.matmul(
                            f_ps,
                            quarters[qi][:, 128 * g : 128 * (g + 1)],
                            fwT_sb[:, qi, :],
                            start=(qi == 0), stop=False)
                    nc.tensor.matmul(
                        f_ps, ones_sb, fb_sb,
                        start=False, stop=True)
                    f_tok = qp.tile([128, D], F16, tag="ftok")
                    nc.scalar.activation(f_tok, f_ps, ACTF.Tanh)
                    fp16m = qp.tile([128, D], F16, tag="fpm")
                    nc.vector.tensor_tensor(fp16m, f_tok, pw_rep_sb, op=ALU.mult)
                    nc.vector.tensor_reduce(
                        p_sb[:, g : g + 1], fp16m.unsqueeze(1), axis=AXL.X,
                        op=ALU.add)
                psig = qp.tile([128, NG], F32, bufs=1)
                nc.scalar.activation(psig, p_sb, ACTF.Sigmoid, bias=pb_sb)
                nc.sync.dma_start(out=p_out_d.ap(), in_=psig)

    nc.compile()
    return nc


def _wrap_idx(flat):
    """token j lives at idxs[j % 128, j // 128] (int32)."""
    arr = np.zeros((128, NG), np.int32)
    j = np.arange(NTOK)
    arr[j % 128, j // 128] = flat.astype(np.int32)
    return arr


def prepare_in_maps(inputs):
    skills = np.asarray(inputs["skills"])
    responses = np.asarray(inputs["responses"])
    x = (skills + V * responses).astype(np.int64)

    Mk = np.asarray(inputs["Mk"], np.float32)
    Mv0 = np.asarray(inputs["Mv0"], np.float32)
    eW = np.asarray(inputs["eW"], np.float32)
    aW = np.asarray(inputs["aW"], np.float32)
    fW = np.asarray(inputs["fW"], np.float32)
    pW = np.asarray(inputs["pW"], np.float32)

    # mv0_sc[8*mq+b, (q,d)] = Mv0[16*q+mq, d]
    mq_i = (np.arange(128) // 8)          # partition -> mq
    mv0_sc = Mv0[(16 * np.arange(NQ)[None, :, None] + mq_i[:, None, None]),
                 np.arange(D)[None, None, :]]
    mv0_sc = mv0_sc.reshape(128, NQ * D).astype(np.float16)
    mask8 = np.zeros((128, BL), np.float16)
    mask8[np.arange(128), np.arange(128) % 8] = 1.0

    common = {
        "k_emb": np.asarray(inputs["k_emb"], np.float16),
        "v_emb": np.asarray(inputs["v_emb"], np.float16),
        "mkT": np.ascontiguousarray(Mk.T).astype(np.float16),
        "ewT": np.ascontiguousarray(eW.T).astype(np.float16),
        "awT": np.ascontiguousarray(aW.T).astype(np.float16),
        "fwT": np.ascontiguousarray(fW.T).astype(np.float16),
        "pw_rep": np.broadcast_to(
            pW.reshape(1, D), (128, D)).astype(np.float16).copy(),
        "eb": np.asarray(inputs["eb"], np.float32).reshape(1, D).astype(np.float16),
        "ab": np.asarray(inputs["ab"], np.float32).reshape(1, D).astype(np.float16),
        "fb": np.asarray(inputs["fb"], np.float32).reshape(1, D).astype(np.float16),
        "pb_rep": np.full(
            (128, 1), float(np.asarray(inputs["pb"]).reshape(-1)[0]), np.float32),
        "mv0": mv0_sc,
        "mask8": mask8,
        "ident": np.eye(128, dtype=np.float16),
        "ones_row": np.ones((1, 128), np.float16),
    }

    in_maps = []
    for c in range(NCORES):
        rows = slice(c * BL, (c + 1) * BL)
        sk = np.zeros((BL, TP), np.int64)
        sk[:, :T] = skills[rows]
        xv = np.zeros((BL, TP), np.int64)
        xv[:, :T] = x[rows]
        m = dict(common)
        m["idx_k"] = _wrap_idx(sk.reshape(-1))
        m["idx_v"] = _wrap_idx(xv.reshape(-1))
        in_maps.append(m)
    return in_maps


_CACHE = {}


def run_on_hw(inputs, trace=False):
    if "nc" not in _CACHE:
        _CACHE["nc"] = build_nc()
    nc = _CACHE["nc"]
    in_maps = prepare_in_maps(inputs)
    res = run_bass_kernel_spmd(
        nc, in_maps, core_ids=list(range(NCORES)), trace=trace)
    outs = []
    for c in range(NCORES):
        outs.append(unscramble(res.results[c]["p_out"])[:, 1:T])
    out = np.concatenate(outs, axis=0).astype(np.float32)
    return out, res


def unscramble(ptok):
    """[128, NG] token-major p -> (BL, TP) batch-major."""
    p = np.zeros((BL, TP), np.float32)
    for b in range(BL):
        p[b, 0:128] = ptok[:, 2 * b]
        p[b, 128:256] = ptok[:, 2 * b + 1]
    return p


def kernel(**inputs):
    out, _ = run_on_hw(inputs, trace=False)
    return out
